# revision 5
# baseline (speedup 1.0000x reference)
# Trainium2 Bass kernel for the 5-branch channel-attention module.
#
# Layout of the computation per batch sample n:
#   avg/max pool of x[n, :, :, TORSO] over (T, torso joints) -> p[c, {avg,max}]
#   h    = relu(W1 @ p + b1)                    (5 branches, HID=16)
#   g    = sigmoid(W2 @ (h_avg + h_max) + 2*b2) (per branch, per channel)
#   out[n, c, t, j] = x[n, c, t, perm[j]] * g[group(j), c]
#
# Sharding: pure data parallel, batch N=64 split over 8 cores (8 samples
# each); the tiny MLP weights are replicated. Each core streams its
# 12.5 MiB x-shard in, does the gating on-chip, and streams 12.5 MiB out.

import numpy as np
from contextlib import ExitStack

import concourse.bass as bass
import concourse.bacc as bacc
import concourse.tile as tile
from concourse import mybir
from concourse.bass_utils import run_bass_kernel_spmd

N, C, T, V = 64, 256, 64, 25
HID = 16
NF = 5
NCORES = 8
NLOC = N // NCORES          # samples per core
NCH = C // 128              # channel chunks of 128 partitions
POOLSZ = T * 5              # elements pooled per channel (T x 5 torso joints)

F32 = mybir.dt.float32

# Output column j takes input column perm[j], scaled by gate of group g.
# Encoded as contiguous runs: (group, src_col, dst_col, n_cols).
RUNS = [
    (0, 0, 0, 4), (0, 20, 4, 1),      # TORSO      [0,1,2,3,20]
    (1, 8, 5, 4), (1, 23, 9, 2),      # LEFT_HAND  [8,9,10,11,23,24]
    (2, 16, 11, 4),                   # LEFT_LEG   [16,17,18,19]
    (3, 4, 15, 4), (3, 21, 19, 2),    # RIGHT_HAND [4,5,6,7,21,22]
    (4, 12, 21, 4),                   # RIGHT_LEG  [12,13,14,15]
]
# Torso pooling source runs.
TRUNS = [(0, 4), (20, 1)]

_CACHE: dict = {}


def _build():
    if "nc" in _CACHE:
        return _CACHE["nc"]

    nc = bacc.Bacc("TRN2", target_bir_lowering=False, debug=False,
                   num_devices=NCORES)

    x = nc.dram_tensor("x", [NLOC, C, T, V], F32, kind="ExternalInput").ap()
    W1s = nc.dram_tensor("W1s", [NF, HID, C], F32, kind="ExternalInput").ap()
    b1s = nc.dram_tensor("b1s", [NF, HID], F32, kind="ExternalInput").ap()
    W2s = nc.dram_tensor("W2s", [NF, C, HID], F32, kind="ExternalInput").ap()
    b2s = nc.dram_tensor("b2s", [NF, C], F32, kind="ExternalInput").ap()
    out = nc.dram_tensor("out", [NLOC, C, T, V], F32, kind="ExternalOutput").ap()

    XY = mybir.AxisListType.XY

    with tile.TileContext(nc) as tc, ExitStack() as ctx:
        cpool = ctx.enter_context(tc.tile_pool(name="const", bufs=1))
        xpool = ctx.enter_context(tc.tile_pool(name="x", bufs=6))
        opool = ctx.enter_context(tc.tile_pool(name="o", bufs=6))
        spool = ctx.enter_context(tc.tile_pool(name="small", bufs=8))
        php = ctx.enter_context(tc.tile_pool(name="ph", bufs=4, space="PSUM"))
        pgp = ctx.enter_context(tc.tile_pool(name="pg", bufs=4, space="PSUM"))

        # ---- replicated constants -------------------------------------
        # w1t[ch][c', f, h] = W1s[f, h, ch*128 + c']
        w1t = []
        for ch in range(NCH):
            t = cpool.tile([128, NF, HID], F32, tag=f"w1_{ch}")
            for f in range(NF):
                nc.sync.dma_start(
                    out=t[:, f, :],
                    in_=W1s.transpose([2, 0, 1])[ch * 128:(ch + 1) * 128, f])
            w1t.append(t)
        # w2t[ch][h, f, c'] = W2s[f, ch*128 + c', h]
        w2t = []
        for ch in range(NCH):
            t = cpool.tile([HID, NF, 128], F32, tag=f"w2_{ch}")
            for f in range(NF):
                nc.sync.dma_start(
                    out=t[:, f, :],
                    in_=W2s.transpose([2, 0, 1])[:, f, ch * 128:(ch + 1) * 128])
            w2t.append(t)
        # b1t[h, f] = b1s[f, h]
        b1t = cpool.tile([HID, NF], F32, tag="b1")
        nc.sync.dma_start(out=b1t[:], in_=b1s.transpose([1, 0]))
        # b2t[ch][c', f] = 2 * b2s[f, ch*128 + c']
        b2t = []
        for ch in range(NCH):
            t = cpool.tile([128, NF], F32, tag=f"b2_{ch}")
            nc.sync.dma_start(
                out=t[:],
                in_=b2s.transpose([1, 0])[ch * 128:(ch + 1) * 128])
            nc.scalar.mul(t[:], t[:], 2.0)
            b2t.append(t)

        # ---- per-sample pipeline --------------------------------------
        for n in range(NLOC):
            xts, pts = [], []
            for ch in range(NCH):
                xt = xpool.tile([128, T, V], F32, tag="xt")
                nc.sync.dma_start(out=xt[:], in_=x[n, ch * 128:(ch + 1) * 128])
                xts.append(xt)

                # avg & max pool over (T, torso joints)
                s1 = spool.tile([128, 1], F32, tag="s1")
                s2 = spool.tile([128, 1], F32, tag="s2")
                m1 = spool.tile([128, 1], F32, tag="m1")
                m2 = spool.tile([128, 1], F32, tag="m2")
                (c0, l0), (c1, l1) = TRUNS
                nc.vector.reduce_sum(out=s1[:], in_=xt[:, :, c0:c0 + l0], axis=XY)
                nc.vector.reduce_sum(out=s2[:], in_=xt[:, :, c1:c1 + l1], axis=XY)
                nc.vector.reduce_max(out=m1[:], in_=xt[:, :, c0:c0 + l0], axis=XY)
                nc.vector.reduce_max(out=m2[:], in_=xt[:, :, c1:c1 + l1], axis=XY)

                p = spool.tile([128, 2], F32, tag="p")
                st = spool.tile([128, 1], F32, tag="st")
                nc.vector.tensor_add(st[:], s1[:], s2[:])
                nc.vector.tensor_scalar_mul(p[:, 0:1], st[:], 1.0 / POOLSZ)
                nc.vector.tensor_max(p[:, 1:2], m1[:], m2[:])
                pts.append(p)

            # layer 1: ph[h, f, j] = sum_c W1s[f,h,c] * p[c, j]
            ph = php.tile([HID, NF, 2], F32, tag="ph")
            for f in range(NF):
                for ch in range(NCH):
                    nc.tensor.matmul(ph[:, f, :], w1t[ch][:, f, :], pts[ch][:],
                                     start=(ch == 0), stop=(ch == NCH - 1))

            # relu(ph + b1), then sum the avg/max halves
            hpre = spool.tile([HID, NF, 2], F32, tag="hpre")
            nc.vector.tensor_add(hpre[:, :, 0], ph[:, :, 0], b1t[:])
            nc.vector.tensor_add(hpre[:, :, 1], ph[:, :, 1], b1t[:])
            hr = spool.tile([HID, NF, 2], F32, tag="hr")
            nc.scalar.activation(hr[:], hpre[:],
                                 mybir.ActivationFunctionType.Relu)
            hs = spool.tile([HID, NF], F32, tag="hs")
            nc.vector.tensor_add(hs[:], hr[:, :, 0], hr[:, :, 1])

            for ch in range(NCH):
                # layer 2: pg[c', f] = sum_h W2s[f,c,h] * hs[h, f]
                pg = pgp.tile([128, NF], F32, tag="pg")
                for f in range(NF):
                    nc.tensor.matmul(pg[:, f:f + 1], w2t[ch][:, f, :],
                                     hs[:, f:f + 1], start=True, stop=True)
                gp = spool.tile([128, NF], F32, tag="gp")
                nc.vector.tensor_add(gp[:], pg[:], b2t[ch][:])
                gate = spool.tile([128, NF], F32, tag="gate")
                nc.scalar.activation(gate[:], gp[:],
                                     mybir.ActivationFunctionType.Sigmoid)

                # gated, column-permuted copy into the output tile;
                # runs are split across ACT / DVE / GpSimd to balance load
                ot = opool.tile([128, T, V], F32, tag="ot")
                for i, (g, s0, d0, ln) in enumerate(RUNS):
                    if i in (2, 4):
                        nc.scalar.activation(ot[:, :, d0:d0 + ln],
                                             xts[ch][:, :, s0:s0 + ln],
                                             mybir.ActivationFunctionType.Copy,
                                             scale=gate[:, g:g + 1])
                    elif i in (1, 5, 7):
                        nc.gpsimd.tensor_scalar_mul(ot[:, :, d0:d0 + ln],
                                                    xts[ch][:, :, s0:s0 + ln],
                                                    gate[:, g:g + 1])
                    else:
                        nc.vector.tensor_scalar_mul(ot[:, :, d0:d0 + ln],
                                                    xts[ch][:, :, s0:s0 + ln],
                                                    gate[:, g:g + 1])
                nc.sync.dma_start(out=out[n, ch * 128:(ch + 1) * 128], in_=ot[:])

    nc.compile()
    _CACHE["nc"] = nc
    return nc


def run(inputs: dict, trace: bool = False, **kw):
    nc = _build()
    x = np.ascontiguousarray(inputs["x"], dtype=np.float32)
    reps = {k: np.ascontiguousarray(inputs[k], dtype=np.float32)
            for k in ("W1s", "b1s", "W2s", "b2s")}
    in_maps = [
        {"x": x[i * NLOC:(i + 1) * NLOC], **reps}
        for i in range(NCORES)
    ]
    res = run_bass_kernel_spmd(nc, in_maps, list(range(NCORES)),
                               trace=trace, **kw)
    full = np.concatenate([res.results[i]["out"] for i in range(NCORES)],
                          axis=0)
    return full.astype(np.float32, copy=False), res


def kernel(**inputs) -> np.ndarray:
    full, _ = run(inputs)
    return full


# revision 7
# speedup vs baseline: 1.6298x; 1.6298x over previous
# Trainium2 Bass kernel for the 5-branch channel-attention module.
#
# Layout of the computation per batch sample n:
#   avg/max pool of x[n, :, :, TORSO] over (T, torso joints) -> p[c, {avg,max}]
#   h    = relu(W1 @ p + b1)                    (5 branches, HID=16)
#   g    = sigmoid(W2 @ (h_avg + h_max) + 2*b2) (per branch, per channel)
#   out[n, c, t, j] = x[n, c, t, perm[j]] * g[group(j), c]
#
# Sharding: pure data parallel, batch N=64 split over 8 cores (8 samples
# each); the tiny MLP weights are replicated. Each core streams its
# 12.5 MiB x-shard in, does the gating on-chip, and streams 12.5 MiB out.

import numpy as np
from contextlib import ExitStack

import concourse.bass as bass
import concourse.bacc as bacc
import concourse.tile as tile
from concourse import mybir
from concourse.bass_utils import run_bass_kernel_spmd

N, C, T, V = 64, 256, 64, 25
HID = 16
NF = 5
NCORES = 8
NLOC = N // NCORES          # samples per core
NCH = C // 128              # channel chunks of 128 partitions
POOLSZ = T * 5              # elements pooled per channel (T x 5 torso joints)

F32 = mybir.dt.float32

# Output column j takes input column perm[j], scaled by gate of group g.
# Encoded as contiguous runs: (group, src_col, dst_col, n_cols).
RUNS = [
    (0, 0, 0, 4), (0, 20, 4, 1),      # TORSO      [0,1,2,3,20]
    (1, 8, 5, 4), (1, 23, 9, 2),      # LEFT_HAND  [8,9,10,11,23,24]
    (2, 16, 11, 4),                   # LEFT_LEG   [16,17,18,19]
    (3, 4, 15, 4), (3, 21, 19, 2),    # RIGHT_HAND [4,5,6,7,21,22]
    (4, 12, 21, 4),                   # RIGHT_LEG  [12,13,14,15]
]
# Torso pooling source runs.
TRUNS = [(0, 4), (20, 1)]

_CACHE: dict = {}


def _build():
    if "nc" in _CACHE:
        return _CACHE["nc"]

    nc = bacc.Bacc("TRN2", target_bir_lowering=False, debug=False,
                   num_devices=NCORES)

    x = nc.dram_tensor("x", [NLOC, C, T, V], F32, kind="ExternalInput").ap()
    W1s = nc.dram_tensor("W1s", [NF, HID, C], F32, kind="ExternalInput").ap()
    b1s = nc.dram_tensor("b1s", [NF, HID], F32, kind="ExternalInput").ap()
    W2s = nc.dram_tensor("W2s", [NF, C, HID], F32, kind="ExternalInput").ap()
    b2s = nc.dram_tensor("b2s", [NF, C], F32, kind="ExternalInput").ap()
    out = nc.dram_tensor("out", [NLOC, C, T, V], F32, kind="ExternalOutput").ap()

    XY = mybir.AxisListType.XY

    with tile.TileContext(nc) as tc, ExitStack() as ctx:
        cpool = ctx.enter_context(tc.tile_pool(name="const", bufs=1))
        xpool = ctx.enter_context(tc.tile_pool(name="x", bufs=6))
        opool = ctx.enter_context(tc.tile_pool(name="o", bufs=6))
        spool = ctx.enter_context(tc.tile_pool(name="small", bufs=8))
        php = ctx.enter_context(tc.tile_pool(name="ph", bufs=4, space="PSUM"))
        pgp = ctx.enter_context(tc.tile_pool(name="pg", bufs=4, space="PSUM"))

        # ---- replicated constants -------------------------------------
        # w1t[ch][c', f, h] = W1s[f, h, ch*128 + c']
        w1t = []
        for ch in range(NCH):
            t = cpool.tile([128, NF, HID], F32, tag=f"w1_{ch}")
            for f in range(NF):
                nc.sync.dma_start(
                    out=t[:, f, :],
                    in_=W1s.transpose([2, 0, 1])[ch * 128:(ch + 1) * 128, f])
            w1t.append(t)
        # w2t[ch][h, f, c'] = W2s[f, ch*128 + c', h]
        w2t = []
        for ch in range(NCH):
            t = cpool.tile([HID, NF, 128], F32, tag=f"w2_{ch}")
            for f in range(NF):
                nc.sync.dma_start(
                    out=t[:, f, :],
                    in_=W2s.transpose([2, 0, 1])[:, f, ch * 128:(ch + 1) * 128])
            w2t.append(t)
        # b1t[h, f] = b1s[f, h]
        b1t = cpool.tile([HID, NF], F32, tag="b1")
        nc.sync.dma_start(out=b1t[:], in_=b1s.transpose([1, 0]))
        # b2t[ch][c', f] = 2 * b2s[f, ch*128 + c']
        b2t = []
        for ch in range(NCH):
            t = cpool.tile([128, NF], F32, tag=f"b2_{ch}")
            nc.sync.dma_start(
                out=t[:],
                in_=b2s.transpose([1, 0])[ch * 128:(ch + 1) * 128])
            nc.scalar.mul(t[:], t[:], 2.0)
            b2t.append(t)

        # ---- per-sample pipeline --------------------------------------
        for n in range(NLOC):
            xts, pts = [], []
            for ch in range(NCH):
                xt = xpool.tile([128, T, V], F32, tag="xt")
                nc.sync.dma_start(out=xt[:], in_=x[n, ch * 128:(ch + 1) * 128])
                xts.append(xt)

                # avg & max pool over (T, torso joints).  The scaled sums
                # run on ACT (activation accum_out folds the 1/320), the
                # max reductions on DVE.
                s1 = spool.tile([128, 1], F32, tag="s1")
                s2 = spool.tile([128, 1], F32, tag="s2")
                m1 = spool.tile([128, 1], F32, tag="m1")
                m2 = spool.tile([128, 1], F32, tag="m2")
                (c0, l0), (c1, l1) = TRUNS
                tr4 = spool.tile([128, T, l0], F32, tag="tr4")
                tr1 = spool.tile([128, T, l1], F32, tag="tr1")
                nc.scalar.activation(tr4[:], xt[:, :, c0:c0 + l0],
                                     mybir.ActivationFunctionType.Copy,
                                     scale=1.0 / POOLSZ, accum_out=s1[:])
                nc.scalar.activation(tr1[:], xt[:, :, c1:c1 + l1],
                                     mybir.ActivationFunctionType.Copy,
                                     scale=1.0 / POOLSZ, accum_out=s2[:])
                nc.vector.reduce_max(out=m1[:], in_=xt[:, :, c0:c0 + l0], axis=XY)
                nc.vector.reduce_max(out=m2[:], in_=xt[:, :, c1:c1 + l1], axis=XY)

                p = spool.tile([128, 2], F32, tag="p")
                nc.vector.tensor_add(p[:, 0:1], s1[:], s2[:])
                nc.vector.tensor_max(p[:, 1:2], m1[:], m2[:])
                pts.append(p)

            # layer 1: ph[h, f, j] = sum_c W1s[f,h,c] * p[c, j]
            ph = php.tile([HID, NF, 2], F32, tag="ph")
            for f in range(NF):
                for ch in range(NCH):
                    nc.tensor.matmul(ph[:, f, :], w1t[ch][:, f, :], pts[ch][:],
                                     start=(ch == 0), stop=(ch == NCH - 1))

            # relu(ph + b1), then sum the avg/max halves
            hpre = spool.tile([HID, NF, 2], F32, tag="hpre")
            nc.vector.tensor_add(hpre[:, :, 0], ph[:, :, 0], b1t[:])
            nc.vector.tensor_add(hpre[:, :, 1], ph[:, :, 1], b1t[:])
            hr = spool.tile([HID, NF, 2], F32, tag="hr")
            nc.scalar.activation(hr[:], hpre[:],
                                 mybir.ActivationFunctionType.Relu)
            hs = spool.tile([HID, NF], F32, tag="hs")
            nc.vector.tensor_add(hs[:], hr[:, :, 0], hr[:, :, 1])

            for ch in range(NCH):
                # layer 2: pg[c', f] = sum_h W2s[f,c,h] * hs[h, f]
                pg = pgp.tile([128, NF], F32, tag="pg")
                for f in range(NF):
                    nc.tensor.matmul(pg[:, f:f + 1], w2t[ch][:, f, :],
                                     hs[:, f:f + 1], start=True, stop=True)
                gp = spool.tile([128, NF], F32, tag="gp")
                nc.vector.tensor_add(gp[:], pg[:], b2t[ch][:])
                gate = spool.tile([128, NF], F32, tag="gate")
                nc.scalar.activation(gate[:], gp[:],
                                     mybir.ActivationFunctionType.Sigmoid)

                # gated, column-permuted copy into the output tile;
                # runs are split across DVE (12 cols) / ACT (13 cols)
                ot = opool.tile([128, T, V], F32, tag="ot")
                for i, (g, s0, d0, ln) in enumerate(RUNS):
                    if i in (1, 2, 4, 7):
                        nc.scalar.activation(ot[:, :, d0:d0 + ln],
                                             xts[ch][:, :, s0:s0 + ln],
                                             mybir.ActivationFunctionType.Copy,
                                             scale=gate[:, g:g + 1])
                    else:
                        nc.vector.tensor_scalar_mul(ot[:, :, d0:d0 + ln],
                                                    xts[ch][:, :, s0:s0 + ln],
                                                    gate[:, g:g + 1])
                nc.sync.dma_start(out=out[n, ch * 128:(ch + 1) * 128], in_=ot[:])

    nc.compile()
    _CACHE["nc"] = nc
    return nc


def run(inputs: dict, trace: bool = False, **kw):
    nc = _build()
    x = np.ascontiguousarray(inputs["x"], dtype=np.float32)
    reps = {k: np.ascontiguousarray(inputs[k], dtype=np.float32)
            for k in ("W1s", "b1s", "W2s", "b2s")}
    in_maps = [
        {"x": x[i * NLOC:(i + 1) * NLOC], **reps}
        for i in range(NCORES)
    ]
    res = run_bass_kernel_spmd(nc, in_maps, list(range(NCORES)),
                               trace=trace, **kw)
    full = np.concatenate([res.results[i]["out"] for i in range(NCORES)],
                          axis=0)
    return full.astype(np.float32, copy=False), res


def kernel(**inputs) -> np.ndarray:
    full, _ = run(inputs)
    return full


# revision 9
# speedup vs baseline: 1.7423x; 1.0690x over previous
# Trainium2 Bass kernel for the 5-branch channel-attention module.
#
# Layout of the computation per batch sample n:
#   avg/max pool of x[n, :, :, TORSO] over (T, torso joints) -> p[c, {avg,max}]
#   h    = relu(W1 @ p + b1)                    (5 branches, HID=16)
#   g    = sigmoid(W2 @ (h_avg + h_max) + 2*b2) (per branch, per channel)
#   out[n, c, t, j] = x[n, c, t, perm[j]] * g[group(j), c]
#
# Sharding: pure data parallel, batch N=64 split over 8 cores (8 samples
# each); the tiny MLP weights are replicated. Each core streams its
# 12.5 MiB x-shard in, does the gating on-chip, and streams 12.5 MiB out.

import numpy as np
from contextlib import ExitStack

import concourse.bass as bass
import concourse.bacc as bacc
import concourse.tile as tile
from concourse import mybir
from concourse.bass_utils import run_bass_kernel_spmd

N, C, T, V = 64, 256, 64, 25
HID = 16
NF = 5
NCORES = 8
NLOC = N // NCORES          # samples per core
NCH = C // 128              # channel chunks of 128 partitions
POOLSZ = T * 5              # elements pooled per channel (T x 5 torso joints)

F32 = mybir.dt.float32

# Output column j takes input column perm[j], scaled by gate of group g.
# Encoded as contiguous runs: (group, src_col, dst_col, n_cols).
RUNS = [
    (0, 0, 0, 4), (0, 20, 4, 1),      # TORSO      [0,1,2,3,20]
    (1, 8, 5, 4), (1, 23, 9, 2),      # LEFT_HAND  [8,9,10,11,23,24]
    (2, 16, 11, 4),                   # LEFT_LEG   [16,17,18,19]
    (3, 4, 15, 4), (3, 21, 19, 2),    # RIGHT_HAND [4,5,6,7,21,22]
    (4, 12, 21, 4),                   # RIGHT_LEG  [12,13,14,15]
]
# Torso pooling source runs.
TRUNS = [(0, 4), (20, 1)]

_CACHE: dict = {}


def _build():
    if "nc" in _CACHE:
        return _CACHE["nc"]

    nc = bacc.Bacc("TRN2", target_bir_lowering=False, debug=False,
                   num_devices=NCORES)

    x = nc.dram_tensor("x", [NLOC, C, T, V], F32, kind="ExternalInput").ap()
    W1s = nc.dram_tensor("W1s", [NF, HID, C], F32, kind="ExternalInput").ap()
    b1s = nc.dram_tensor("b1s", [NF, HID], F32, kind="ExternalInput").ap()
    W2s = nc.dram_tensor("W2s", [NF, C, HID], F32, kind="ExternalInput").ap()
    b2s = nc.dram_tensor("b2s", [NF, C], F32, kind="ExternalInput").ap()
    out = nc.dram_tensor("out", [NLOC, C, T, V], F32, kind="ExternalOutput").ap()

    XY = mybir.AxisListType.XY

    with tile.TileContext(nc) as tc, ExitStack() as ctx:
        cpool = ctx.enter_context(tc.tile_pool(name="const", bufs=1))
        xpool = ctx.enter_context(tc.tile_pool(name="x", bufs=8))
        opool = ctx.enter_context(tc.tile_pool(name="o", bufs=8))
        spool = ctx.enter_context(tc.tile_pool(name="small", bufs=12))
        php = ctx.enter_context(tc.tile_pool(name="ph", bufs=4, space="PSUM"))
        pgp = ctx.enter_context(tc.tile_pool(name="pg", bufs=4, space="PSUM"))

        # ---- replicated constants -------------------------------------
        # w1t[ch][c', f, h] = W1s[f, h, ch*128 + c']
        w1t = []
        for ch in range(NCH):
            t = cpool.tile([128, NF, HID], F32, tag=f"w1_{ch}")
            for f in range(NF):
                nc.gpsimd.dma_start(
                    out=t[:, f, :],
                    in_=W1s.transpose([2, 0, 1])[ch * 128:(ch + 1) * 128, f])
            w1t.append(t)
        # w2t[ch][h, f, c'] = W2s[f, ch*128 + c', h]
        w2t = []
        for ch in range(NCH):
            t = cpool.tile([HID, NF, 128], F32, tag=f"w2_{ch}")
            for f in range(NF):
                nc.scalar.dma_start(
                    out=t[:, f, :],
                    in_=W2s.transpose([2, 0, 1])[:, f, ch * 128:(ch + 1) * 128])
            w2t.append(t)
        # b1t[h, f] = b1s[f, h]
        b1t = cpool.tile([HID, NF], F32, tag="b1")
        nc.gpsimd.dma_start(out=b1t[:], in_=b1s.transpose([1, 0]))
        # b2t[ch][c', f] = 2 * b2s[f, ch*128 + c']
        b2t = []
        for ch in range(NCH):
            t = cpool.tile([128, NF], F32, tag=f"b2_{ch}")
            nc.scalar.dma_start(
                out=t[:],
                in_=b2s.transpose([1, 0])[ch * 128:(ch + 1) * 128])
            nc.scalar.mul(t[:], t[:], 2.0)
            b2t.append(t)

        # ---- per-sample pipeline --------------------------------------
        for n in range(NLOC):
            xts, pts = [], []
            for ch in range(NCH):
                xt = xpool.tile([128, T, V], F32, tag="xt")
                nc.sync.dma_start(out=xt[:], in_=x[n, ch * 128:(ch + 1) * 128])
                xts.append(xt)

                # avg & max pool over (T, torso joints).  The scaled sums
                # run on ACT (activation accum_out folds the 1/320), the
                # max reductions on DVE.
                s1 = spool.tile([128, 1], F32, tag="s1")
                s2 = spool.tile([128, 1], F32, tag="s2")
                m1 = spool.tile([128, 1], F32, tag="m1")
                m2 = spool.tile([128, 1], F32, tag="m2")
                (c0, l0), (c1, l1) = TRUNS
                tr4 = spool.tile([128, T, l0], F32, tag="tr4")
                tr1 = spool.tile([128, T, l1], F32, tag="tr1")
                nc.scalar.activation(tr4[:], xt[:, :, c0:c0 + l0],
                                     mybir.ActivationFunctionType.Copy,
                                     scale=1.0 / POOLSZ, accum_out=s1[:])
                nc.scalar.activation(tr1[:], xt[:, :, c1:c1 + l1],
                                     mybir.ActivationFunctionType.Copy,
                                     scale=1.0 / POOLSZ, accum_out=s2[:])
                nc.vector.reduce_max(out=m1[:], in_=xt[:, :, c0:c0 + l0], axis=XY)
                nc.vector.reduce_max(out=m2[:], in_=xt[:, :, c1:c1 + l1], axis=XY)

                p = spool.tile([128, 2], F32, tag="p")
                nc.vector.tensor_add(p[:, 0:1], s1[:], s2[:])
                nc.vector.tensor_max(p[:, 1:2], m1[:], m2[:])
                pts.append(p)

            # layer 1: ph[h, f, j] = sum_c W1s[f,h,c] * p[c, j]
            ph = php.tile([HID, NF, 2], F32, tag="ph")
            for f in range(NF):
                for ch in range(NCH):
                    nc.tensor.matmul(ph[:, f, :], w1t[ch][:, f, :], pts[ch][:],
                                     start=(ch == 0), stop=(ch == NCH - 1))

            # relu(ph + b1), then sum the avg/max halves
            hpre = spool.tile([HID, NF, 2], F32, tag="hpre")
            nc.vector.tensor_add(hpre[:, :, 0], ph[:, :, 0], b1t[:])
            nc.vector.tensor_add(hpre[:, :, 1], ph[:, :, 1], b1t[:])
            hr = spool.tile([HID, NF, 2], F32, tag="hr")
            nc.scalar.activation(hr[:], hpre[:],
                                 mybir.ActivationFunctionType.Relu)
            hs = spool.tile([HID, NF], F32, tag="hs")
            nc.vector.tensor_add(hs[:], hr[:, :, 0], hr[:, :, 1])

            for ch in range(NCH):
                # layer 2: pg[c', f] = sum_h W2s[f,c,h] * hs[h, f]
                pg = pgp.tile([128, NF], F32, tag="pg")
                for f in range(NF):
                    nc.tensor.matmul(pg[:, f:f + 1], w2t[ch][:, f, :],
                                     hs[:, f:f + 1], start=True, stop=True)
                gp = spool.tile([128, NF], F32, tag="gp")
                nc.vector.tensor_add(gp[:], pg[:], b2t[ch][:])
                gate = spool.tile([128, NF], F32, tag="gate")
                nc.scalar.activation(gate[:], gp[:],
                                     mybir.ActivationFunctionType.Sigmoid)

                # gated, column-permuted copy into the output tile;
                # runs are split across DVE (12 cols) / ACT (13 cols)
                ot = opool.tile([128, T, V], F32, tag="ot")
                for i, (g, s0, d0, ln) in enumerate(RUNS):
                    if i in (1, 2, 4, 7):
                        nc.scalar.activation(ot[:, :, d0:d0 + ln],
                                             xts[ch][:, :, s0:s0 + ln],
                                             mybir.ActivationFunctionType.Copy,
                                             scale=gate[:, g:g + 1])
                    else:
                        nc.vector.tensor_scalar_mul(ot[:, :, d0:d0 + ln],
                                                    xts[ch][:, :, s0:s0 + ln],
                                                    gate[:, g:g + 1])
                nc.gpsimd.dma_start(out=out[n, ch * 128:(ch + 1) * 128], in_=ot[:])

    nc.compile()
    _CACHE["nc"] = nc
    return nc


def run(inputs: dict, trace: bool = False, **kw):
    nc = _build()
    x = np.ascontiguousarray(inputs["x"], dtype=np.float32)
    reps = {k: np.ascontiguousarray(inputs[k], dtype=np.float32)
            for k in ("W1s", "b1s", "W2s", "b2s")}
    in_maps = [
        {"x": x[i * NLOC:(i + 1) * NLOC], **reps}
        for i in range(NCORES)
    ]
    res = run_bass_kernel_spmd(nc, in_maps, list(range(NCORES)),
                               trace=trace, **kw)
    full = np.concatenate([res.results[i]["out"] for i in range(NCORES)],
                          axis=0)
    return full.astype(np.float32, copy=False), res


def kernel(**inputs) -> np.ndarray:
    full, _ = run(inputs)
    return full


# revision 13
# speedup vs baseline: 2.2360x; 1.2834x over previous
# Trainium2 Bass kernel for the 5-branch channel-attention module.
#
# Layout of the computation per batch sample n:
#   avg/max pool of x[n, :, :, TORSO] over (T, torso joints) -> p[c, {avg,max}]
#   h    = relu(W1 @ p + b1)                    (5 branches, HID=16)
#   g    = sigmoid(W2 @ (h_avg + h_max) + 2*b2) (per branch, per channel)
#   out[n, c, t, j] = x[n, c, t, perm[j]] * g[group(j), c]
#
# Sharding: pure data parallel, batch N=64 split over 8 cores (8 samples
# each); the tiny MLP weights are replicated. Each core streams its
# 12.5 MiB x-shard in, does the gating on-chip, and streams 12.5 MiB out.

import numpy as np
from contextlib import ExitStack

import concourse.bass as bass
import concourse.bacc as bacc
import concourse.tile as tile
from concourse import masks, mybir
from concourse.bass_utils import run_bass_kernel_spmd

N, C, T, V = 64, 256, 64, 25
HID = 16
NF = 5
NCORES = 8
NLOC = N // NCORES          # samples per core
NCH = C // 128              # channel chunks of 128 partitions
POOLSZ = T * 5              # elements pooled per channel (T x 5 torso joints)

F32 = mybir.dt.float32

# Output column j takes input column perm[j], scaled by gate of group g.
# Encoded as contiguous runs: (group, src_col, dst_col, n_cols).
RUNS = [
    (0, 0, 0, 4), (0, 20, 4, 1),      # TORSO      [0,1,2,3,20]
    (1, 8, 5, 4), (1, 23, 9, 2),      # LEFT_HAND  [8,9,10,11,23,24]
    (2, 16, 11, 4),                   # LEFT_LEG   [16,17,18,19]
    (3, 4, 15, 4), (3, 21, 19, 2),    # RIGHT_HAND [4,5,6,7,21,22]
    (4, 12, 21, 4),                   # RIGHT_LEG  [12,13,14,15]
]
# Torso pooling source runs.
TRUNS = [(0, 4), (20, 1)]

_CACHE: dict = {}


def _build():
    if "nc" in _CACHE:
        return _CACHE["nc"]

    nc = bacc.Bacc("TRN2", target_bir_lowering=False, debug=False,
                   num_devices=NCORES)

    x = nc.dram_tensor("x", [NLOC, C, T, V], F32, kind="ExternalInput").ap()
    W1s = nc.dram_tensor("W1s", [NF, HID, C], F32, kind="ExternalInput").ap()
    b1s = nc.dram_tensor("b1s", [NF, HID], F32, kind="ExternalInput").ap()
    W2s = nc.dram_tensor("W2s", [NF, C, HID], F32, kind="ExternalInput").ap()
    b2s = nc.dram_tensor("b2s", [NF, C], F32, kind="ExternalInput").ap()
    out = nc.dram_tensor("out", [NLOC, C, T, V], F32, kind="ExternalOutput").ap()

    XY = mybir.AxisListType.XY

    with tile.TileContext(nc) as tc, ExitStack() as ctx:
        cpool = ctx.enter_context(tc.tile_pool(name="const", bufs=1))
        xpool = ctx.enter_context(tc.tile_pool(name="x", bufs=8))
        opool = ctx.enter_context(tc.tile_pool(name="o", bufs=8))
        spool = ctx.enter_context(tc.tile_pool(name="small", bufs=12))
        php = ctx.enter_context(tc.tile_pool(name="ph", bufs=2, space="PSUM"))
        pgp = ctx.enter_context(tc.tile_pool(name="pg", bufs=3, space="PSUM"))
        tpp = ctx.enter_context(tc.tile_pool(name="tp", bufs=2, space="PSUM"))

        # ---- replicated constants -------------------------------------
        # All weight loads are contiguous DMAs in natural layout; the
        # required transposes run on the (otherwise idle) PE so the DMA
        # rings never see element-granularity descriptors.
        ident = cpool.tile([128, 128], F32, tag="ident")
        masks.make_identity(nc, ident[:])

        w1nat = cpool.tile([NF * HID, C], F32, tag="w1nat")
        nc.scalar.dma_start(out=w1nat[:], in_=W1s.flatten_outer_dims())
        w2nat = []
        for ch in range(NCH):
            t = cpool.tile([128, NF, HID], F32, tag=f"w2nat_{ch}")
            nc.scalar.dma_start(
                out=t[:],
                in_=W2s.transpose([1, 0, 2])[ch * 128:(ch + 1) * 128])
            w2nat.append(t)
        b1nat = cpool.tile([NF, HID], F32, tag="b1nat")
        nc.scalar.dma_start(out=b1nat[:], in_=b1s[:])
        b2nat = cpool.tile([NF, C], F32, tag="b2nat")
        nc.scalar.dma_start(out=b2nat[:], in_=b2s[:])

        # w1t[ch][c', f*16+h] = W1s[f, h, ch*128 + c']
        w1t = []
        for ch in range(NCH):
            pt = tpp.tile([128, 128], F32, tag="tp")
            nc.tensor.transpose(pt[:, 0:NF * HID],
                                w1nat[:, ch * 128:(ch + 1) * 128],
                                ident[0:NF * HID, 0:NF * HID])
            t = cpool.tile([128, NF * HID], F32, tag=f"w1_{ch}")
            nc.vector.tensor_copy(t[:], pt[:, 0:NF * HID])
            w1t.append(t)
        # w2t[ch][h, f, c'] = W2s[f, ch*128 + c', h]
        w2t = []
        for ch in range(NCH):
            t = cpool.tile([HID, NF, 128], F32, tag=f"w2_{ch}")
            for f in range(NF):
                pt = tpp.tile([128, 128], F32, tag="tp")
                nc.tensor.transpose(pt[0:HID, :], w2nat[ch][:, f, :], ident[:])
                nc.vector.tensor_copy(t[:, f, :], pt[0:HID, :])
            w2t.append(t)
        # b1t[h, f] = b1s[f, h]
        pt = tpp.tile([128, 128], F32, tag="tp")
        nc.tensor.transpose(pt[0:HID, 0:NF], b1nat[:], ident[0:NF, 0:NF])
        b1t = cpool.tile([HID, NF], F32, tag="b1")
        nc.vector.tensor_copy(b1t[:], pt[0:HID, 0:NF])
        # b2t[ch][c', f] = 2 * b2s[f, ch*128 + c']
        b2t = []
        for ch in range(NCH):
            pt = tpp.tile([128, 128], F32, tag="tp")
            nc.tensor.transpose(pt[:, 0:NF], b2nat[:, ch * 128:(ch + 1) * 128],
                                ident[0:NF, 0:NF])
            t = cpool.tile([128, NF], F32, tag=f"b2_{ch}")
            nc.scalar.mul(t[:], pt[:, 0:NF], 2.0)
            b2t.append(t)

        # ---- per-sample pipeline --------------------------------------
        for n in range(NLOC):
            xts, pts = [], []
            for ch in range(NCH):
                xt = xpool.tile([128, T, V], F32, tag="xt")
                nc.sync.dma_start(out=xt[:], in_=x[n, ch * 128:(ch + 1) * 128])
                xts.append(xt)

                # avg & max pool over (T, torso joints).  The scaled sums
                # run on ACT (activation accum_out folds the 1/320), the
                # max reductions on DVE.
                s1 = spool.tile([128, 1], F32, tag="s1")
                s2 = spool.tile([128, 1], F32, tag="s2")
                m1 = spool.tile([128, 1], F32, tag="m1")
                m2 = spool.tile([128, 1], F32, tag="m2")
                (c0, l0), (c1, l1) = TRUNS
                tr4 = spool.tile([128, T, l0], F32, tag="tr4")
                tr1 = spool.tile([128, T, l1], F32, tag="tr1")
                nc.scalar.activation(tr4[:], xt[:, :, c0:c0 + l0],
                                     mybir.ActivationFunctionType.Copy,
                                     scale=1.0 / POOLSZ, accum_out=s1[:])
                nc.scalar.activation(tr1[:], xt[:, :, c1:c1 + l1],
                                     mybir.ActivationFunctionType.Copy,
                                     scale=1.0 / POOLSZ, accum_out=s2[:])
                nc.vector.reduce_max(out=m1[:], in_=xt[:, :, c0:c0 + l0], axis=XY)
                nc.vector.reduce_max(out=m2[:], in_=xt[:, :, c1:c1 + l1], axis=XY)

                p = spool.tile([128, 2], F32, tag="p")
                nc.vector.tensor_add(p[:, 0:1], s1[:], s2[:])
                nc.vector.tensor_max(p[:, 1:2], m1[:], m2[:])
                pts.append(p)

            # layer 1: ph[h, f, j] = sum_c W1s[f,h,c] * p[c, j]
            ph = php.tile([HID, NF, 2], F32, tag="ph")
            for f in range(NF):
                for ch in range(NCH):
                    nc.tensor.matmul(ph[:, f, :],
                                     w1t[ch][:, f * HID:(f + 1) * HID],
                                     pts[ch][:],
                                     start=(ch == 0), stop=(ch == NCH - 1))

            # relu(ph + b1), then sum the avg/max halves
            hpre = spool.tile([HID, NF, 2], F32, tag="hpre")
            nc.vector.tensor_add(hpre[:, :, 0], ph[:, :, 0], b1t[:])
            nc.vector.tensor_add(hpre[:, :, 1], ph[:, :, 1], b1t[:])
            hr = spool.tile([HID, NF, 2], F32, tag="hr")
            nc.scalar.activation(hr[:], hpre[:],
                                 mybir.ActivationFunctionType.Relu)
            hs = spool.tile([HID, NF], F32, tag="hs")
            nc.vector.tensor_add(hs[:], hr[:, :, 0], hr[:, :, 1])

            for ch in range(NCH):
                # layer 2: pg[c', f] = sum_h W2s[f,c,h] * hs[h, f]
                pg = pgp.tile([128, NF], F32, tag="pg")
                for f in range(NF):
                    nc.tensor.matmul(pg[:, f:f + 1], w2t[ch][:, f, :],
                                     hs[:, f:f + 1], start=True, stop=True)
                gp = spool.tile([128, NF], F32, tag="gp")
                nc.vector.tensor_add(gp[:], pg[:], b2t[ch][:])
                gate = spool.tile([128, NF], F32, tag="gate")
                nc.scalar.activation(gate[:], gp[:],
                                     mybir.ActivationFunctionType.Sigmoid)

                # gated, column-permuted copy into the output tile;
                # runs are split across DVE (12 cols) / ACT (13 cols)
                ot = opool.tile([128, T, V], F32, tag="ot")
                for i, (g, s0, d0, ln) in enumerate(RUNS):
                    if i in (1, 2, 4, 7):
                        nc.scalar.activation(ot[:, :, d0:d0 + ln],
                                             xts[ch][:, :, s0:s0 + ln],
                                             mybir.ActivationFunctionType.Copy,
                                             scale=gate[:, g:g + 1])
                    else:
                        nc.vector.tensor_scalar_mul(ot[:, :, d0:d0 + ln],
                                                    xts[ch][:, :, s0:s0 + ln],
                                                    gate[:, g:g + 1])
                nc.gpsimd.dma_start(out=out[n, ch * 128:(ch + 1) * 128], in_=ot[:])

    nc.compile()
    _CACHE["nc"] = nc
    return nc


def run(inputs: dict, trace: bool = False, **kw):
    nc = _build()
    x = np.ascontiguousarray(inputs["x"], dtype=np.float32)
    reps = {k: np.ascontiguousarray(inputs[k], dtype=np.float32)
            for k in ("W1s", "b1s", "W2s", "b2s")}
    in_maps = [
        {"x": x[i * NLOC:(i + 1) * NLOC], **reps}
        for i in range(NCORES)
    ]
    res = run_bass_kernel_spmd(nc, in_maps, list(range(NCORES)),
                               trace=trace, **kw)
    full = np.concatenate([res.results[i]["out"] for i in range(NCORES)],
                          axis=0)
    return full.astype(np.float32, copy=False), res


def kernel(**inputs) -> np.ndarray:
    full, _ = run(inputs)
    return full
